# revision 10
# baseline (speedup 1.0000x reference)
"""Cross-Scale Non-Local Attention kernel for 8x Trainium2 NeuronCores.

Data-parallel over batch: each of the 8 cores processes one sample
(B=8, H=W=64, C=64). Per-core Bass/Tile program:

  1. x loaded in 4 chunks; each chunk is PE-transposed to channel-major
     xT [c=64, 4096] with g/theta matmuls interleaved per chunk so the
     tensor engine stays dense (HAM stays released at 2.4 GHz).
  2. g [pix, 64] = prelu(xT.T @ g_w), bounced to DRAM as bf16 g_poly
     [18,18,4,256] (polyphase layout, zero border ring = conv-transpose
     SAME padding); the 18 shifted dynamic-filter views kg[q,qw,kb]
     [n=128, 4, 256] are gathered back by strided DMA.
  3. thetaT = prelu(theta_w.T @ xT) into a zero-padded 66x66 buffer.
  4. phi from 4-tap bilinear downsample; 3x3 patches in the padded
     [18,18] domain; the softmax scale 10/max(||phi_patch||,1e-6) is
     folded directly into phi_patchT (linear), so scoresT comes out
     pre-scaled and Exp needs no per-partition scale operand.
  5. Per pixel-chunk ch: scoresT [n, pix] via 9 shifted-window matmuls
     (K=32); E = exp(score) written to attnT bf16 (no max subtraction -
     |score| < 80 so fp32 exp cannot overflow); 6S = sixes.T @ E via
     matmul; attnT *= 1/(6S) broadcast (folds both softmax denom and
     the final /6).
  6. Deconv as polyphase conv-transpose, one chunk behind the scores
     loop, oriented so PSUM comes out output-major (no PE transposes):
     psum[pix 128, rc 512] += attnT_window[n, 2, 64].T @ kg[q,qw,kb][:, 2rh:2rh+2, :]
     over 18 shifts; drain = one scalar copy + 2 output DMAs with 1 KiB
     descriptors.

All matmuls use float32r (FP22 multiply, FP32 accumulate) or bf16 at
full PE rate. bf16 warmup/keepalive matmuls bridge the PE-idle windows
(initial x DMA, phi-norm chain) so the HAM clock gate stays at 8/8.
"""

import numpy as np

_CACHE = {}

# Problem constants (hardcoded per harness contract)
B = 8
H = W = 64
C = 64
CI = 32
HS = WS = 16
N = 256          # HS*WS low-res positions
PH = 66          # padded attn/theta spatial extent (64 + 1 halo each side)


def _build_nc():
    import concourse.bass as bass
    import concourse.tile as tile
    from concourse import bacc, mybir
    from concourse.masks import make_identity
    from contextlib import ExitStack

    F32 = mybir.dt.float32
    F32R = mybir.dt.float32r
    BF16 = mybir.dt.bfloat16
    Alu = mybir.AluOpType
    Act = mybir.ActivationFunctionType

    def r_(ap):
        return ap.bitcast(F32R)

    nc = bacc.Bacc("TRN2", debug=False)

    x_h = nc.dram_tensor("x", [H, W, C], F32, kind="ExternalInput")
    thw_h = nc.dram_tensor("theta_w", [C, CI], F32, kind="ExternalInput")
    thb_h = nc.dram_tensor("theta_b", [CI], F32, kind="ExternalInput")
    tha_h = nc.dram_tensor("theta_alpha", [CI], F32, kind="ExternalInput")
    phw_h = nc.dram_tensor("phi_w", [C, CI], F32, kind="ExternalInput")
    phb_h = nc.dram_tensor("phi_b", [CI], F32, kind="ExternalInput")
    pha_h = nc.dram_tensor("phi_alpha", [CI], F32, kind="ExternalInput")
    gw_h = nc.dram_tensor("g_w", [C, C], F32, kind="ExternalInput")
    gb_h = nc.dram_tensor("g_b", [C], F32, kind="ExternalInput")
    ga_h = nc.dram_tensor("g_alpha", [C], F32, kind="ExternalInput")
    y_h = nc.dram_tensor("y", [4 * H, 4 * W, C], F32, kind="ExternalOutput")

    with tile.TileContext(nc) as tc, ExitStack() as top:
        ec = top.enter_context

        consts = ec(tc.tile_pool(name="consts", bufs=1))
        xp_pool = ec(tc.tile_pool(name="xp_pool", bufs=1))
        persist = ec(tc.tile_pool(name="persist", bufs=1))
        phip = ec(tc.tile_pool(name="phip", bufs=1))
        dramp = ec(tc.tile_pool(name="dramp", bufs=1, space="DRAM"))
        staging = ec(tc.tile_pool(name="staging", bufs=3))
        ps_misc = ec(tc.tile_pool(name="ps_misc", bufs=2, space="PSUM"))
        ps_sc = ec(tc.tile_pool(name="ps_sc", bufs=2, space="PSUM"))
        ps_d = ec(tc.tile_pool(name="ps_d", bufs=2, space="PSUM"))
        ps_tr = ec(tc.tile_pool(name="ps_tr", bufs=2, space="PSUM"))

        # ---- constants / weights in SBUF ----
        ident = consts.tile([128, 128], F32)
        make_identity(nc, ident)
        # x chunks issued before anything else on the DMA queues
        xP = xp_pool.tile([128, 32, C], F32)
        x_r = x_h.ap().rearrange("h w c -> (h w) c").rearrange(
            "(t p) c -> p t c", p=128)
        for xc in range(4):
            nc.sync.dma_start(
                out=xP[:, xc * 8:(xc + 1) * 8, :],
                in_=x_r[:, xc * 8:(xc + 1) * 8, :])
        # HAM warmup: bf16 matmuls keep the PE busy through the initial
        # x-load DMA so the clock gate stays open when real work starts.
        wu = consts.tile([128, 512], BF16)
        nc.vector.memset(wu, 0.0)
        ps_wu = ps_sc.tile([128, 512], F32, tag="sc", name="ps_wu")
        for i in range(16):
            nc.tensor.matmul(ps_wu, wu[:, :128], wu, start=True, stop=True)
        thw_sb = consts.tile([C, CI], F32)
        nc.sync.dma_start(out=r_(thw_sb), in_=r_(thw_h.ap()))
        phw_sb = consts.tile([C, CI], F32)
        nc.sync.dma_start(out=r_(phw_sb), in_=r_(phw_h.ap()))
        gw_sb = consts.tile([C, C], F32)
        nc.sync.dma_start(out=r_(gw_sb), in_=r_(gw_h.ap()))
        thb_sb = consts.tile([CI, 1], F32)
        nc.sync.dma_start(out=thb_sb, in_=thb_h.ap().unsqueeze(1))
        tha_sb = consts.tile([CI, 1], F32)
        nc.sync.dma_start(out=tha_sb, in_=tha_h.ap().unsqueeze(1))
        phb_sb = consts.tile([CI, 1], F32)
        nc.sync.dma_start(out=phb_sb, in_=phb_h.ap().unsqueeze(1))
        pha_sb = consts.tile([CI, 1], F32)
        nc.sync.dma_start(out=pha_sb, in_=pha_h.ap().unsqueeze(1))
        gb_row = consts.tile([1, C], F32)
        nc.sync.dma_start(out=gb_row, in_=gb_h.ap().unsqueeze(0))
        ga_row = consts.tile([1, C], F32)
        nc.sync.dma_start(out=ga_row, in_=ga_h.ap().unsqueeze(0))
        gb_bc = consts.tile([128, C], F32)
        nc.gpsimd.partition_broadcast(gb_bc, gb_row)
        ga_bc = consts.tile([128, C], F32)
        nc.gpsimd.partition_broadcast(ga_bc, ga_row)
        z66 = consts.tile([128, PH], F32)
        nc.vector.memset(z66, 0.0)
        sixes128 = consts.tile([128, 1], BF16)
        nc.vector.memset(sixes128, 6.0)
        ones32 = consts.tile([CI, 1], F32)
        nc.vector.memset(ones32, 1.0)
        ztb16 = consts.tile([128, 1024], BF16)
        nc.vector.memset(ztb16, 0.0)

        # ---- persistent activation buffers ----
        thetaT_pad = persist.tile([CI, PH, PH], F32)
        nc.vector.tensor_copy(out=r_(thetaT_pad[:, 0, :]), in_=z66[:CI])
        nc.vector.tensor_copy(out=r_(thetaT_pad[:, PH - 1, :]), in_=z66[:CI])
        nc.vector.tensor_copy(out=r_(thetaT_pad[:, :, 0]), in_=z66[:CI])
        nc.vector.tensor_copy(out=r_(thetaT_pad[:, :, PH - 1]), in_=z66[:CI])
        attnT = persist.tile([128, 2, PH, PH], BF16)
        for kb in range(2):
            nc.vector.tensor_copy(out=attnT[:, kb, 0, :], in_=z66)
            nc.vector.tensor_copy(out=attnT[:, kb, PH - 1, :], in_=z66)
            nc.vector.tensor_copy(out=attnT[:, kb, :, 0], in_=z66)
            nc.vector.tensor_copy(out=attnT[:, kb, :, PH - 1], in_=z66)
        phi_patchT = persist.tile([CI, 3, 3, N], F32)

        phiT_pad = phip.tile([CI, 18, 18], F32)
        nc.vector.memset(phiT_pad, 0.0)
        n2p = phip.tile([1, 324], F32)
        nrm = phip.tile([1, N], F32)
        phi_inT = phip.tile([C, HS, WS], F32)
        s10b = phip.tile([128, N], F32)

        # polyphase layout: g_poly[hq, wq, hr, (wr c)] = g[4hq+hr-4, 4wq+wr-4, c]
        g_poly = dramp.tile([18, 18, 4, 256], BF16)
        g_lin = dramp.tile([H, W, C], BF16)

        with ExitStack() as st1:
            e1 = st1.enter_context
            xt_pool = e1(tc.tile_pool(name="xt_pool", bufs=1))
            gsb_pool = e1(tc.tile_pool(name="gsb_pool", bufs=1))
            ttmp = e1(tc.tile_pool(name="ttmp", bufs=2))
            gtmp = e1(tc.tile_pool(name="gtmp", bufs=3))

            xT = xt_pool.tile([C, H, W], F32)
            xTf = xT.rearrange("c h w -> c (h w)")
            g_sb = gsb_pool.tile([128, 32, C], BF16)

            def theta_chunk(ch):
                h0 = ch * 8
                ps_t = ps_misc.tile([CI, 512], F32, tag="m", name=f"ps_t{ch}")
                nc.tensor.matmul(
                    ps_t, r_(thw_sb), r_(xTf[:, ch * 512:(ch + 1) * 512]),
                    start=True, stop=True)
                t_lin = ttmp.tile([CI, 8, W], F32, tag="tl")
                nc.vector.tensor_scalar_add(
                    t_lin.rearrange("p a b -> p (a b)"), ps_t, thb_sb)
                t_neg = ttmp.tile([CI, 8, W], F32, tag="tn")
                nc.vector.tensor_scalar(
                    t_neg.rearrange("p a b -> p (a b)"),
                    t_lin.rearrange("p a b -> p (a b)"),
                    0.0, tha_sb, Alu.min, Alu.mult)
                nc.vector.scalar_tensor_tensor(
                    out=r_(thetaT_pad[:, 1 + h0:9 + h0, 1:65]),
                    in0=t_lin, scalar=0.0, in1=t_neg,
                    op0=Alu.max, op1=Alu.add)

            # interleaved: transposes -> g matmuls -> theta per x chunk
            for xc in range(4):
                for t in range(xc * 8, (xc + 1) * 8):
                    ps_x = ps_misc.tile([C, 128], F32, tag="m",
                                        name=f"ps_x{t}")
                    nc.tensor.transpose(ps_x, xP[:, t, :], ident)
                    nc.scalar.copy(
                        out=r_(xTf[:, t * 128:(t + 1) * 128]), in_=ps_x)
                for t in range(xc * 8, (xc + 1) * 8):
                    ps_g = ps_misc.tile([128, C], F32, tag="m",
                                        name=f"ps_g{t}")
                    nc.tensor.matmul(
                        ps_g, r_(xTf[:, t * 128:(t + 1) * 128]), r_(gw_sb),
                        start=True, stop=True)
                    gv = gtmp.tile([128, C], F32, tag="gv")
                    nc.vector.tensor_add(gv, ps_g, gb_bc)
                    gm1 = gtmp.tile([128, C], F32, tag="gm1")
                    nc.vector.tensor_scalar_max(gm1, gv, 0.0)
                    nc.vector.tensor_scalar_min(gv, gv, 0.0)
                    nc.vector.tensor_mul(gv, gv, ga_bc)
                    nc.vector.scalar_tensor_tensor(
                        out=g_sb[:, t, :], in0=gm1, scalar=1.0, in1=gv,
                        op0=Alu.mult, op1=Alu.add)
                theta_chunk(2 * xc)
                theta_chunk(2 * xc + 1)

            # g_sb -> g_lin [h, w, c], then 4 DRAM->DRAM repacks into the
            # polyphase interior (one per hr phase)
            glint = g_lin.rearrange("(t a) w c -> a w t c", a=2)
            for p1 in range(2):
                nc.sync.dma_start(
                    out=glint[p1], in_=g_sb[p1 * 64:(p1 + 1) * 64, :, :])
            gl5 = g_lin.rearrange("(hq hr) (wq wr) c -> hq hr wq (wr c)",
                                  hr=4, wr=4)
            for hr in range(4):
                nc.sync.dma_start(out=g_poly[1:17, 1:17, hr, :],
                                  in_=gl5[:, hr, :, :])

            # phi: bilinear downsample (4-tap avg) then 1x1 conv + prelu
            xv = xT.rearrange("c (hq hs) (wq ws) -> c hq hs wq ws", hs=4, ws=4)
            nc.vector.tensor_add(r_(phi_inT), xv[:, :, 1, :, 1],
                                 xv[:, :, 1, :, 2])
            nc.vector.tensor_add(r_(phi_inT), phi_inT, xv[:, :, 2, :, 1])
            nc.vector.tensor_add(r_(phi_inT), phi_inT, xv[:, :, 2, :, 2])
            nc.vector.tensor_scalar_mul(r_(phi_inT), phi_inT, 0.25)
            ps_phi = ps_misc.tile([CI, N], F32, tag="m")
            nc.tensor.matmul(
                ps_phi, r_(phw_sb), r_(phi_inT.rearrange("c a b -> c (a b)")),
                start=True, stop=True)
            p_lin = ttmp.tile([CI, HS, WS], F32, tag="pl")
            nc.vector.tensor_scalar_add(
                p_lin.rearrange("p a b -> p (a b)"), ps_phi, phb_sb)
            p_neg = ttmp.tile([CI, HS, WS], F32, tag="pn")
            nc.vector.tensor_scalar(
                p_neg.rearrange("p a b -> p (a b)"),
                p_lin.rearrange("p a b -> p (a b)"),
                0.0, pha_sb, Alu.min, Alu.mult)
            nc.vector.scalar_tensor_tensor(
                out=phiT_pad[:, 1:17, 1:17],
                in0=p_lin, scalar=0.0, in1=p_neg,
                op0=Alu.max, op1=Alu.add)
            # keepalive: cover the patch/square DVE stretch
            for i in range(8):
                nc.tensor.matmul(ps_wu, wu[:, :128], wu,
                                 start=True, stop=True)

            # phi patches (padded windows) + per-patch L2 norm; the
            # softmax scale s10 = 10/max(norm,1e-6) is folded into the
            # patches so Exp consumes pre-scaled scores.
            for kh in range(3):
                for kw in range(3):
                    nc.vector.tensor_copy(
                        out=r_(phi_patchT[:, kh, kw, :].rearrange(
                            "p (a b) -> p a b", b=WS)),
                        in_=phiT_pad[:, kh:kh + 16, kw:kw + 16])
            sq = ttmp.tile([CI, 324], F32, tag="sq")
            nc.scalar.activation(r_(sq),
                                 phiT_pad.rearrange("p a b -> p (a b)"),
                                 Act.Square)
            ps_n2 = ps_misc.tile([1, 324], F32, tag="m")
            nc.tensor.matmul(ps_n2, r_(ones32), r_(sq), start=True, stop=True)
            nc.scalar.copy(out=n2p, in_=ps_n2)
            n2v = n2p.rearrange("p (a b) -> p a b", b=18)
            nrm3 = nrm.rearrange("p (a b) -> p a b", b=WS)
            nc.vector.tensor_add(nrm3, n2v[:, 0:16, 0:16], n2v[:, 0:16, 1:17])
            for kh in range(3):
                for kw in range(3):
                    if kh == 0 and kw < 2:
                        continue
                    nc.vector.tensor_add(
                        nrm3, nrm3, n2v[:, kh:kh + 16, kw:kw + 16])
            nc.scalar.sqrt(nrm, nrm)
            nc.vector.tensor_scalar_max(nrm, nrm, 1e-6)
            nc.vector.reciprocal(nrm, nrm)
            nc.vector.tensor_scalar_mul(nrm, nrm, 10.0)
            nc.gpsimd.partition_broadcast(s10b, nrm)
            for kh in range(3):
                for kw in range(3):
                    nc.vector.tensor_mul(
                        r_(phi_patchT[:, kh, kw, :]),
                        phi_patchT[:, kh, kw, :], s10b[:CI])
            # keepalive: bridge the phi-norm chain so HAM stays at 8/8
            for i in range(16):
                nc.tensor.matmul(ps_wu, wu[:, :128], wu,
                                 start=True, stop=True)

        # ---- stage 2: fused scores/softmax/deconv pipeline ----
        with ExitStack() as st2:
            e2 = st2.enter_context
            kgp = e2(tc.tile_pool(name="kgp", bufs=1))
            rbp = e2(tc.tile_pool(name="rbp", bufs=3))
            schp = e2(tc.tile_pool(name="schp", bufs=2))
            trp = e2(tc.tile_pool(name="trp", bufs=3))

            # zero the g_poly border ring (conv-transpose SAME padding);
            # issued here so these DMAs never gate the stage-1 pipeline
            gp_r0 = g_poly[0].rearrange("b r x -> (b r x)").rearrange(
                "(p f) -> p f", f=512)
            nc.sync.dma_start(out=gp_r0, in_=ztb16[:36, :512])
            gp_r1 = g_poly[17].rearrange("b r x -> (b r x)").rearrange(
                "(p f) -> p f", f=512)
            nc.sync.dma_start(out=gp_r1, in_=ztb16[:36, :512])
            gp_c0 = g_poly[1:17, 0].rearrange("a r x -> a (r x)")
            nc.sync.dma_start(out=gp_c0, in_=ztb16[:16])
            gp_c1 = g_poly[1:17, 17].rearrange("a r x -> a (r x)")
            nc.sync.dma_start(out=gp_c1, in_=ztb16[:16])

            # gather the 18 dynamic-filter tiles from g_poly, one DMA each
            # kg[q,qw,kb][(i,j), r, (rw c)] = g_poly[i+kb*8+q, j+qw, r, :]
            kg = {}
            for q in range(3):
                for qw in range(3):
                    for kb in range(2):
                        t_ = kgp.tile([128, 4, 256], BF16,
                                      tag=f"kg{q}{qw}{kb}",
                                      name=f"kg{q}{qw}{kb}")
                        gsrc = g_poly[kb * 8 + q: kb * 8 + q + 8,
                                      qw: qw + 16, :, :]
                        nc.sync.dma_start(out=t_, in_=gsrc)
                        kg[(q, qw, kb)] = t_

            yr = y_h.ap().rearrange(
                "(M r) (Mw w) c -> M Mw r w c", r=4, w=4)
            pending = [None]

            def drain(pend):
                tr_in, pc, u = pend
                ps_t2 = ps_tr.tile([128, 512], F32, tag="tt",
                                   name=f"ps_tr{pc}_{u}")
                for k in range(4):
                    nc.tensor.transpose(
                        ps_t2[:, k * 128:(k + 1) * 128],
                        tr_in[:, k * 128:(k + 1) * 128], ident)
                st_ = staging.tile([128, 512], F32, tag="stg",
                                   name=f"st{pc}_{u}")
                nc.scalar.copy(out=st_, in_=ps_t2)
                st3 = st_.rearrange("p (k rw c) -> p k rw c", k=4, rw=2)
                rr = u // 2
                w0 = 2 * (u % 2)
                for k in range(4):
                    for p1 in range(2):
                        nc.sync.dma_start(
                            out=yr[pc * 8 + 2 * k + p1, :, rr, w0:w0 + 2, :],
                            in_=st3[p1 * 64:(p1 + 1) * 64, k])

            def deconv_pc(pc):
                h0 = pc * 8
                for u in range(8):
                    ps_o = ps_d.tile([128, 512], F32, tag="d",
                                     name=f"ps_o{pc}_{u}")
                    first = True
                    for q in range(3):
                        for qw in range(3):
                            for kb in range(2):
                                nc.tensor.matmul(
                                    ps_o,
                                    kg[(q, qw, kb)].rearrange(
                                        "p r x -> p (r x)")[:, u * 128:
                                                            (u + 1) * 128],
                                    attnT[:, kb, h0 + 2 - q:h0 + 10 - q,
                                          2 - qw:66 - qw],
                                    start=first,
                                    stop=(q == 2 and qw == 2 and kb == 1))
                                first = False
                    tr_in = trp.tile([128, 512], F32, tag="ti",
                                     name=f"ti{pc}_{u}")
                    nc.scalar.copy(out=tr_in, in_=ps_o)
                    if pending[0] is not None:
                        drain(pending[0])
                    pending[0] = (tr_in, pc, u)

            for ch in range(8):
                h0 = ch * 8
                # scoresT for both n-blocks, then E = exp(score)
                for kb in range(2):
                    ps_s = ps_sc.tile([128, 512], F32, tag="sc",
                                      name=f"ps_s{ch}_{kb}")
                    first = True
                    for kh in range(3):
                        for kw in range(3):
                            nc.tensor.matmul(
                                ps_s,
                                r_(phi_patchT[:, kh, kw,
                                              kb * 128:(kb + 1) * 128]),
                                r_(thetaT_pad[:, h0 + kh:h0 + kh + 8,
                                              kw:kw + 64]),
                                start=first, stop=(kh == 2 and kw == 2))
                            first = False
                    nc.scalar.activation(
                        out=attnT[:, kb, 1 + h0:9 + h0, 1:65],
                        in_=ps_s.rearrange("p (a b) -> p a b", b=64),
                        func=Act.Exp)
                if ch >= 2:
                    deconv_pc(ch - 2)
                # 6S = sixes.T @ E (ones-matmul), rb = 1/(6S) broadcast
                ps_S = ps_misc.tile([1, 512], F32, tag="m", name=f"ps_S{ch}")
                for kb in range(2):
                    nc.tensor.matmul(
                        ps_S, sixes128,
                        attnT[:, kb, 1 + h0:9 + h0, 1:65],
                        start=(kb == 0), stop=(kb == 1))
                sch = schp.tile([1, 512], F32, tag="sch", name=f"sch{ch}")
                nc.vector.reciprocal(sch, ps_S)
                rb_t = rbp.tile([128, 512], F32, tag="rb", name=f"rb{ch}")
                nc.gpsimd.partition_broadcast(rb_t, sch)
                rb3 = rb_t.rearrange("p (a b) -> p a b", b=64)
                for kb in range(2):
                    nc.vector.tensor_mul(
                        attnT[:, kb, 1 + h0:9 + h0, 1:65],
                        attnT[:, kb, 1 + h0:9 + h0, 1:65], rb3)
            deconv_pc(6)
            deconv_pc(7)
            drain(pending[0])

    nc.finalize()
    return nc


def kernel(**inputs):
    from concourse.bass_utils import run_bass_kernel_spmd

    if "nc" not in _CACHE:
        _CACHE["nc"] = _build_nc()
    nc = _CACHE["nc"]

    arrs = {k: np.ascontiguousarray(np.asarray(v, dtype=np.float32))
            for k, v in inputs.items()}
    x = arrs.pop("x")
    in_maps = [dict(arrs, x=x[b]) for b in range(B)]
    res = run_bass_kernel_spmd(nc, in_maps, core_ids=list(range(B)))
    return np.stack([res.results[b]["y"] for b in range(B)])


# revision 11
# speedup vs baseline: 1.0632x; 1.0632x over previous
"""Cross-Scale Non-Local Attention kernel for 8x Trainium2 NeuronCores.

Data-parallel over batch: each of the 8 cores processes one sample
(B=8, H=W=64, C=64). Per-core Bass/Tile program:

  1. x loaded in 4 chunks; each chunk is PE-transposed to channel-major
     xT [c=64, 4096] with g/theta matmuls interleaved per chunk so the
     tensor engine stays dense (HAM stays released at 2.4 GHz).
  2. g [pix, 64] = prelu(xT.T @ g_w), bounced to DRAM as bf16 g_poly
     [18,18,4,256] (polyphase layout, zero border ring = conv-transpose
     SAME padding); the 18 shifted dynamic-filter views kg[q,qw,kb]
     [n=128, 4, 256] are gathered back by strided DMA.
  3. thetaT = prelu(theta_w.T @ xT) into a zero-padded 66x66 buffer.
  4. phi from 4-tap bilinear downsample; 3x3 patches in the padded
     [18,18] domain; the softmax scale 10/max(||phi_patch||,1e-6) is
     folded directly into phi_patchT (linear), so scoresT comes out
     pre-scaled and Exp needs no per-partition scale operand.
  5. Per pixel-chunk ch: scoresT [n, pix] via 9 shifted-window matmuls
     (K=32); E = exp(score) written to attnT bf16 (no max subtraction -
     |score| < 80 so fp32 exp cannot overflow); 6S = sixes.T @ E via
     matmul; attnT *= 1/(6S) broadcast (folds both softmax denom and
     the final /6).
  6. Deconv as polyphase conv-transpose, one chunk behind the scores
     loop, oriented so PSUM comes out output-major (no PE transposes):
     psum[pix 128, rc 512] += attnT_window[n, 2, 64].T @ kg[q,qw,kb][:, 2rh:2rh+2, :]
     over 18 shifts; drain = one scalar copy + 2 output DMAs with 1 KiB
     descriptors.

All matmuls use float32r (FP22 multiply, FP32 accumulate) or bf16 at
full PE rate. bf16 warmup/keepalive matmuls bridge the PE-idle windows
(initial x DMA, phi-norm chain) so the HAM clock gate stays at 8/8.
"""

import numpy as np

_CACHE = {}

# Problem constants (hardcoded per harness contract)
B = 8
H = W = 64
C = 64
CI = 32
HS = WS = 16
N = 256          # HS*WS low-res positions
PH = 66          # padded attn/theta spatial extent (64 + 1 halo each side)


def _build_nc():
    import concourse.bass as bass
    import concourse.tile as tile
    from concourse import bacc, mybir
    from concourse.masks import make_identity
    from contextlib import ExitStack

    F32 = mybir.dt.float32
    F32R = mybir.dt.float32r
    BF16 = mybir.dt.bfloat16
    Alu = mybir.AluOpType
    Act = mybir.ActivationFunctionType

    def r_(ap):
        return ap.bitcast(F32R)

    nc = bacc.Bacc("TRN2", debug=False)

    x_h = nc.dram_tensor("x", [H, W, C], F32, kind="ExternalInput")
    thw_h = nc.dram_tensor("theta_w", [C, CI], F32, kind="ExternalInput")
    thb_h = nc.dram_tensor("theta_b", [CI], F32, kind="ExternalInput")
    tha_h = nc.dram_tensor("theta_alpha", [CI], F32, kind="ExternalInput")
    phw_h = nc.dram_tensor("phi_w", [C, CI], F32, kind="ExternalInput")
    phb_h = nc.dram_tensor("phi_b", [CI], F32, kind="ExternalInput")
    pha_h = nc.dram_tensor("phi_alpha", [CI], F32, kind="ExternalInput")
    gw_h = nc.dram_tensor("g_w", [C, C], F32, kind="ExternalInput")
    gb_h = nc.dram_tensor("g_b", [C], F32, kind="ExternalInput")
    ga_h = nc.dram_tensor("g_alpha", [C], F32, kind="ExternalInput")
    y_h = nc.dram_tensor("y", [4 * H, 4 * W, C], F32, kind="ExternalOutput")

    with tile.TileContext(nc) as tc, ExitStack() as top:
        ec = top.enter_context

        consts = ec(tc.tile_pool(name="consts", bufs=1))
        xp_pool = ec(tc.tile_pool(name="xp_pool", bufs=1))
        persist = ec(tc.tile_pool(name="persist", bufs=1))
        phip = ec(tc.tile_pool(name="phip", bufs=1))
        dramp = ec(tc.tile_pool(name="dramp", bufs=1, space="DRAM"))
        staging = ec(tc.tile_pool(name="staging", bufs=3))
        ps_misc = ec(tc.tile_pool(name="ps_misc", bufs=2, space="PSUM"))
        ps_sc = ec(tc.tile_pool(name="ps_sc", bufs=2, space="PSUM"))
        ps_d = ec(tc.tile_pool(name="ps_d", bufs=2, space="PSUM"))
        ps_tr = ec(tc.tile_pool(name="ps_tr", bufs=2, space="PSUM"))

        # ---- constants / weights in SBUF ----
        ident = consts.tile([128, 128], F32)
        make_identity(nc, ident)
        # x chunks issued before anything else on the DMA queues
        xP = xp_pool.tile([128, 32, C], F32)
        x_r = x_h.ap().rearrange("h w c -> (h w) c").rearrange(
            "(t p) c -> p t c", p=128)
        for xc in range(4):
            nc.sync.dma_start(
                out=xP[:, xc * 8:(xc + 1) * 8, :],
                in_=x_r[:, xc * 8:(xc + 1) * 8, :])
        # HAM warmup: bf16 matmuls keep the PE busy through the initial
        # x-load DMA so the clock gate stays open when real work starts.
        wu = consts.tile([128, 512], BF16)
        nc.vector.memset(wu, 0.0)
        ps_wu = ps_sc.tile([128, 512], F32, tag="sc", name="ps_wu")
        for i in range(16):
            nc.tensor.matmul(ps_wu, wu[:, :128], wu, start=True, stop=True)
        thw_sb = consts.tile([C, CI], F32)
        nc.sync.dma_start(out=r_(thw_sb), in_=r_(thw_h.ap()))
        phw_sb = consts.tile([C, CI], F32)
        nc.sync.dma_start(out=r_(phw_sb), in_=r_(phw_h.ap()))
        gw_sb = consts.tile([C, C], F32)
        nc.sync.dma_start(out=r_(gw_sb), in_=r_(gw_h.ap()))
        thb_sb = consts.tile([CI, 1], F32)
        nc.sync.dma_start(out=thb_sb, in_=thb_h.ap().unsqueeze(1))
        tha_sb = consts.tile([CI, 1], F32)
        nc.sync.dma_start(out=tha_sb, in_=tha_h.ap().unsqueeze(1))
        phb_sb = consts.tile([CI, 1], F32)
        nc.sync.dma_start(out=phb_sb, in_=phb_h.ap().unsqueeze(1))
        pha_sb = consts.tile([CI, 1], F32)
        nc.sync.dma_start(out=pha_sb, in_=pha_h.ap().unsqueeze(1))
        gb_row = consts.tile([1, C], F32)
        nc.sync.dma_start(out=gb_row, in_=gb_h.ap().unsqueeze(0))
        ga_row = consts.tile([1, C], F32)
        nc.sync.dma_start(out=ga_row, in_=ga_h.ap().unsqueeze(0))
        gb_bc = consts.tile([128, C], F32)
        nc.gpsimd.partition_broadcast(gb_bc, gb_row)
        ga_bc = consts.tile([128, C], F32)
        nc.gpsimd.partition_broadcast(ga_bc, ga_row)
        z66 = consts.tile([128, PH], F32)
        nc.vector.memset(z66, 0.0)
        sixes128 = consts.tile([128, 1], BF16)
        nc.vector.memset(sixes128, 6.0)
        ones32 = consts.tile([CI, 1], F32)
        nc.vector.memset(ones32, 1.0)
        ztb16 = consts.tile([128, 1024], BF16)
        nc.vector.memset(ztb16, 0.0)

        # ---- persistent activation buffers ----
        thetaT_pad = persist.tile([CI, PH, PH], F32)
        nc.vector.tensor_copy(out=r_(thetaT_pad[:, 0, :]), in_=z66[:CI])
        nc.vector.tensor_copy(out=r_(thetaT_pad[:, PH - 1, :]), in_=z66[:CI])
        nc.vector.tensor_copy(out=r_(thetaT_pad[:, :, 0]), in_=z66[:CI])
        nc.vector.tensor_copy(out=r_(thetaT_pad[:, :, PH - 1]), in_=z66[:CI])
        attnT = persist.tile([128, 2, PH, PH], BF16)
        for kb in range(2):
            nc.vector.tensor_copy(out=attnT[:, kb, 0, :], in_=z66)
            nc.vector.tensor_copy(out=attnT[:, kb, PH - 1, :], in_=z66)
            nc.vector.tensor_copy(out=attnT[:, kb, :, 0], in_=z66)
            nc.vector.tensor_copy(out=attnT[:, kb, :, PH - 1], in_=z66)
        phi_patchT = persist.tile([CI, 3, 3, N], F32)

        phiT_pad = phip.tile([CI, 18, 18], F32)
        nc.vector.memset(phiT_pad, 0.0)
        n2p = phip.tile([1, 324], F32)
        nrm = phip.tile([1, N], F32)
        phi_inT = phip.tile([C, HS, WS], F32)
        s10b = phip.tile([128, N], F32)

        # polyphase layout: g_poly[hq, wq, hr, (wr c)] = g[4hq+hr-4, 4wq+wr-4, c]
        g_poly = dramp.tile([18, 18, 4, 256], BF16)
        g_lin = dramp.tile([H, W, C], BF16)

        with ExitStack() as st1:
            e1 = st1.enter_context
            xt_pool = e1(tc.tile_pool(name="xt_pool", bufs=1))
            gsb_pool = e1(tc.tile_pool(name="gsb_pool", bufs=1))
            ttmp = e1(tc.tile_pool(name="ttmp", bufs=2))
            gtmp = e1(tc.tile_pool(name="gtmp", bufs=3))

            xT = xt_pool.tile([C, H, W], F32)
            xTf = xT.rearrange("c h w -> c (h w)")
            g_sb = gsb_pool.tile([128, 32, C], BF16)

            def theta_chunk(ch):
                h0 = ch * 8
                ps_t = ps_misc.tile([CI, 512], F32, tag="m", name=f"ps_t{ch}")
                nc.tensor.matmul(
                    ps_t, r_(thw_sb), r_(xTf[:, ch * 512:(ch + 1) * 512]),
                    start=True, stop=True)
                t_lin = ttmp.tile([CI, 8, W], F32, tag="tl")
                nc.vector.tensor_scalar_add(
                    t_lin.rearrange("p a b -> p (a b)"), ps_t, thb_sb)
                t_neg = ttmp.tile([CI, 8, W], F32, tag="tn")
                nc.vector.tensor_scalar(
                    t_neg.rearrange("p a b -> p (a b)"),
                    t_lin.rearrange("p a b -> p (a b)"),
                    0.0, tha_sb, Alu.min, Alu.mult)
                nc.vector.scalar_tensor_tensor(
                    out=r_(thetaT_pad[:, 1 + h0:9 + h0, 1:65]),
                    in0=t_lin, scalar=0.0, in1=t_neg,
                    op0=Alu.max, op1=Alu.add)

            # interleaved: transposes -> g matmuls -> theta per x chunk
            for xc in range(4):
                for t in range(xc * 8, (xc + 1) * 8):
                    ps_x = ps_misc.tile([C, 128], F32, tag="m",
                                        name=f"ps_x{t}")
                    nc.tensor.transpose(ps_x, xP[:, t, :], ident)
                    nc.scalar.copy(
                        out=r_(xTf[:, t * 128:(t + 1) * 128]), in_=ps_x)
                for t in range(xc * 8, (xc + 1) * 8):
                    ps_g = ps_misc.tile([128, C], F32, tag="m",
                                        name=f"ps_g{t}")
                    nc.tensor.matmul(
                        ps_g, r_(xTf[:, t * 128:(t + 1) * 128]), r_(gw_sb),
                        start=True, stop=True)
                    gv = gtmp.tile([128, C], F32, tag="gv")
                    nc.vector.tensor_add(gv, ps_g, gb_bc)
                    gm1 = gtmp.tile([128, C], F32, tag="gm1")
                    nc.vector.tensor_scalar_max(gm1, gv, 0.0)
                    nc.vector.tensor_scalar_min(gv, gv, 0.0)
                    nc.vector.tensor_mul(gv, gv, ga_bc)
                    nc.vector.scalar_tensor_tensor(
                        out=g_sb[:, t, :], in0=gm1, scalar=1.0, in1=gv,
                        op0=Alu.mult, op1=Alu.add)
                theta_chunk(2 * xc)
                theta_chunk(2 * xc + 1)

            # g_sb -> g_lin [h, w, c], then 4 DRAM->DRAM repacks into the
            # polyphase interior (one per hr phase)
            glint = g_lin.rearrange("(t a) w c -> a w t c", a=2)
            for p1 in range(2):
                nc.sync.dma_start(
                    out=glint[p1], in_=g_sb[p1 * 64:(p1 + 1) * 64, :, :])
            gl5 = g_lin.rearrange("(hq hr) (wq wr) c -> hq hr wq (wr c)",
                                  hr=4, wr=4)
            for hr in range(4):
                nc.sync.dma_start(out=g_poly[1:17, 1:17, hr, :],
                                  in_=gl5[:, hr, :, :])

            # phi: bilinear downsample (4-tap avg) then 1x1 conv + prelu
            xv = xT.rearrange("c (hq hs) (wq ws) -> c hq hs wq ws", hs=4, ws=4)
            nc.vector.tensor_add(r_(phi_inT), xv[:, :, 1, :, 1],
                                 xv[:, :, 1, :, 2])
            nc.vector.tensor_add(r_(phi_inT), phi_inT, xv[:, :, 2, :, 1])
            nc.vector.tensor_add(r_(phi_inT), phi_inT, xv[:, :, 2, :, 2])
            nc.vector.tensor_scalar_mul(r_(phi_inT), phi_inT, 0.25)
            ps_phi = ps_misc.tile([CI, N], F32, tag="m")
            nc.tensor.matmul(
                ps_phi, r_(phw_sb), r_(phi_inT.rearrange("c a b -> c (a b)")),
                start=True, stop=True)
            p_lin = ttmp.tile([CI, HS, WS], F32, tag="pl")
            nc.vector.tensor_scalar_add(
                p_lin.rearrange("p a b -> p (a b)"), ps_phi, phb_sb)
            p_neg = ttmp.tile([CI, HS, WS], F32, tag="pn")
            nc.vector.tensor_scalar(
                p_neg.rearrange("p a b -> p (a b)"),
                p_lin.rearrange("p a b -> p (a b)"),
                0.0, pha_sb, Alu.min, Alu.mult)
            nc.vector.scalar_tensor_tensor(
                out=phiT_pad[:, 1:17, 1:17],
                in0=p_lin, scalar=0.0, in1=p_neg,
                op0=Alu.max, op1=Alu.add)
            # keepalive: cover the patch/square DVE stretch
            for i in range(8):
                nc.tensor.matmul(ps_wu, wu[:, :128], wu,
                                 start=True, stop=True)

            # phi patches (padded windows) + per-patch L2 norm; the
            # softmax scale s10 = 10/max(norm,1e-6) is folded into the
            # patches so Exp consumes pre-scaled scores.
            for kh in range(3):
                for kw in range(3):
                    nc.vector.tensor_copy(
                        out=r_(phi_patchT[:, kh, kw, :].rearrange(
                            "p (a b) -> p a b", b=WS)),
                        in_=phiT_pad[:, kh:kh + 16, kw:kw + 16])
            sq = ttmp.tile([CI, 324], F32, tag="sq")
            nc.scalar.activation(r_(sq),
                                 phiT_pad.rearrange("p a b -> p (a b)"),
                                 Act.Square)
            ps_n2 = ps_misc.tile([1, 324], F32, tag="m")
            nc.tensor.matmul(ps_n2, r_(ones32), r_(sq), start=True, stop=True)
            nc.scalar.copy(out=n2p, in_=ps_n2)
            n2v = n2p.rearrange("p (a b) -> p a b", b=18)
            nrm3 = nrm.rearrange("p (a b) -> p a b", b=WS)
            nc.vector.tensor_add(nrm3, n2v[:, 0:16, 0:16], n2v[:, 0:16, 1:17])
            for kh in range(3):
                for kw in range(3):
                    if kh == 0 and kw < 2:
                        continue
                    nc.vector.tensor_add(
                        nrm3, nrm3, n2v[:, kh:kh + 16, kw:kw + 16])
            nc.scalar.sqrt(nrm, nrm)
            nc.vector.tensor_scalar_max(nrm, nrm, 1e-6)
            nc.vector.reciprocal(nrm, nrm)
            nc.vector.tensor_scalar_mul(nrm, nrm, 10.0)
            nc.gpsimd.partition_broadcast(s10b, nrm)
            for kh in range(3):
                for kw in range(3):
                    nc.vector.tensor_mul(
                        r_(phi_patchT[:, kh, kw, :]),
                        phi_patchT[:, kh, kw, :], s10b[:CI])
            # keepalive: bridge the phi-norm chain so HAM stays at 8/8
            for i in range(16):
                nc.tensor.matmul(ps_wu, wu[:, :128], wu,
                                 start=True, stop=True)

        # ---- stage 2: fused scores/softmax/deconv pipeline ----
        with ExitStack() as st2:
            e2 = st2.enter_context
            kgp = e2(tc.tile_pool(name="kgp", bufs=1))
            rbp = e2(tc.tile_pool(name="rbp", bufs=3))
            schp = e2(tc.tile_pool(name="schp", bufs=2))
            trp = e2(tc.tile_pool(name="trp", bufs=3))

            # zero the g_poly border ring (conv-transpose SAME padding);
            # issued here so these DMAs never gate the stage-1 pipeline
            gp_r0 = g_poly[0].rearrange("b r x -> (b r x)").rearrange(
                "(p f) -> p f", f=512)
            nc.sync.dma_start(out=gp_r0, in_=ztb16[:36, :512])
            gp_r1 = g_poly[17].rearrange("b r x -> (b r x)").rearrange(
                "(p f) -> p f", f=512)
            nc.sync.dma_start(out=gp_r1, in_=ztb16[:36, :512])
            gp_c0 = g_poly[1:17, 0].rearrange("a r x -> a (r x)")
            nc.sync.dma_start(out=gp_c0, in_=ztb16[:16])
            gp_c1 = g_poly[1:17, 17].rearrange("a r x -> a (r x)")
            nc.sync.dma_start(out=gp_c1, in_=ztb16[:16])

            # gather the 18 dynamic-filter tiles from g_poly, one DMA each
            # kg[q,qw,kb][(i,j), r, (rw c)] = g_poly[i+kb*8+q, j+qw, r, :]
            kg = {}
            for q in range(3):
                for qw in range(3):
                    for kb in range(2):
                        t_ = kgp.tile([128, 4, 256], BF16,
                                      tag=f"kg{q}{qw}{kb}",
                                      name=f"kg{q}{qw}{kb}")
                        gsrc = g_poly[kb * 8 + q: kb * 8 + q + 8,
                                      qw: qw + 16, :, :]
                        nc.sync.dma_start(out=t_, in_=gsrc)
                        kg[(q, qw, kb)] = t_

            # y rows decompose as (pc k p1 r): row = pc*8 + 2k + p1; one
            # DMA per p1-half covers all 4 k-blocks of a drain (the sync
            # engine issues descriptors serially at ~600ns each, so DMA
            # count -- not bytes -- is what this drain must minimize)
            yr3 = y_h.ap().rearrange(
                "(pc k p1 r) (Mw w) c -> pc p1 r Mw k w c",
                k=4, p1=2, r=4, w=4)
            pending = [None]

            def drain(pend):
                tr_in, pc, u = pend
                ps_t2 = ps_tr.tile([128, 512], F32, tag="tt",
                                   name=f"ps_tr{pc}_{u}")
                for k in range(4):
                    nc.tensor.transpose(
                        ps_t2[:, k * 128:(k + 1) * 128],
                        tr_in[:, k * 128:(k + 1) * 128], ident)
                st_ = staging.tile([128, 512], F32, tag="stg",
                                   name=f"st{pc}_{u}")
                nc.scalar.copy(out=st_, in_=ps_t2)
                st3 = st_.rearrange("p (k rw c) -> p k rw c", k=4, rw=2)
                rr = u // 2
                w0 = 2 * (u % 2)
                for p1 in range(2):
                    nc.sync.dma_start(
                        out=yr3[pc, p1, rr, :, :, w0:w0 + 2, :],
                        in_=st3[p1 * 64:(p1 + 1) * 64])

            def deconv_pc(pc):
                h0 = pc * 8
                for u in range(8):
                    ps_o = ps_d.tile([128, 512], F32, tag="d",
                                     name=f"ps_o{pc}_{u}")
                    first = True
                    for q in range(3):
                        for qw in range(3):
                            for kb in range(2):
                                nc.tensor.matmul(
                                    ps_o,
                                    kg[(q, qw, kb)].rearrange(
                                        "p r x -> p (r x)")[:, u * 128:
                                                            (u + 1) * 128],
                                    attnT[:, kb, h0 + 2 - q:h0 + 10 - q,
                                          2 - qw:66 - qw],
                                    start=first,
                                    stop=(q == 2 and qw == 2 and kb == 1))
                                first = False
                    tr_in = trp.tile([128, 512], F32, tag="ti",
                                     name=f"ti{pc}_{u}")
                    nc.scalar.copy(out=tr_in, in_=ps_o)
                    if pending[0] is not None:
                        drain(pending[0])
                    pending[0] = (tr_in, pc, u)

            for ch in range(8):
                h0 = ch * 8
                # scoresT for both n-blocks, then E = exp(score)
                for kb in range(2):
                    ps_s = ps_sc.tile([128, 512], F32, tag="sc",
                                      name=f"ps_s{ch}_{kb}")
                    first = True
                    for kh in range(3):
                        for kw in range(3):
                            nc.tensor.matmul(
                                ps_s,
                                r_(phi_patchT[:, kh, kw,
                                              kb * 128:(kb + 1) * 128]),
                                r_(thetaT_pad[:, h0 + kh:h0 + kh + 8,
                                              kw:kw + 64]),
                                start=first, stop=(kh == 2 and kw == 2))
                            first = False
                    nc.scalar.activation(
                        out=attnT[:, kb, 1 + h0:9 + h0, 1:65],
                        in_=ps_s.rearrange("p (a b) -> p a b", b=64),
                        func=Act.Exp)
                if ch >= 2:
                    deconv_pc(ch - 2)
                # 6S = sixes.T @ E (ones-matmul), rb = 1/(6S) broadcast
                ps_S = ps_misc.tile([1, 512], F32, tag="m", name=f"ps_S{ch}")
                for kb in range(2):
                    nc.tensor.matmul(
                        ps_S, sixes128,
                        attnT[:, kb, 1 + h0:9 + h0, 1:65],
                        start=(kb == 0), stop=(kb == 1))
                sch = schp.tile([1, 512], F32, tag="sch", name=f"sch{ch}")
                nc.vector.reciprocal(sch, ps_S)
                rb_t = rbp.tile([128, 512], F32, tag="rb", name=f"rb{ch}")
                nc.gpsimd.partition_broadcast(rb_t, sch)
                rb3 = rb_t.rearrange("p (a b) -> p a b", b=64)
                for kb in range(2):
                    nc.vector.tensor_mul(
                        attnT[:, kb, 1 + h0:9 + h0, 1:65],
                        attnT[:, kb, 1 + h0:9 + h0, 1:65], rb3)
            deconv_pc(6)
            deconv_pc(7)
            drain(pending[0])

    nc.finalize()
    return nc


def kernel(**inputs):
    from concourse.bass_utils import run_bass_kernel_spmd

    if "nc" not in _CACHE:
        _CACHE["nc"] = _build_nc()
    nc = _CACHE["nc"]

    arrs = {k: np.ascontiguousarray(np.asarray(v, dtype=np.float32))
            for k, v in inputs.items()}
    x = arrs.pop("x")
    in_maps = [dict(arrs, x=x[b]) for b in range(B)]
    res = run_bass_kernel_spmd(nc, in_maps, core_ids=list(range(B)))
    return np.stack([res.results[b]["y"] for b in range(B)])


# revision 13
# speedup vs baseline: 1.0893x; 1.0245x over previous
"""Cross-Scale Non-Local Attention kernel for 8x Trainium2 NeuronCores.

Data-parallel over batch: each of the 8 cores processes one sample
(B=8, H=W=64, C=64). Per-core Bass/Tile program:

  1. x loaded in 4 chunks; each chunk is PE-transposed to channel-major
     xT [c=64, 4096] with g/theta matmuls interleaved per chunk so the
     tensor engine stays dense.
  2. g [pix, 64] = prelu(xT.T @ g_w), bounced to DRAM as bf16 g_poly
     [18,18,4,256] (polyphase layout, zero border ring = conv-transpose
     SAME padding); the 18 shifted dynamic-filter views kg[q,qw,kb]
     [n=128, 4, 256] are gathered back by strided DMA.
  3. theta/phi 1x1 convs run with 4x-replicated weights [64,128] so the
     PE emits 4 identical copies on partition groups; the prelu writes
     then bake per-tap spatial shifts into packed operands:
     thetaPackA/B [128=(4 tap, 32 ci), 64, 64] (+ C for the 9th tap).
     Scores need only 3 matmuls (K=128,128,32) instead of 9 K=32 ones.
  4. phi tap packs [128=(tap,ci), n] are extracted from a 4x-replicated
     padded phi plane by partition-aligned shifted-window copies; the
     softmax scale 10/max(||phi_patch||,1e-6) is folded into the packs
     (linear), so Exp needs no per-partition scale operand.
  5. Per pixel-chunk ch: scoresT [n, pix] via the 3 packed matmuls;
     E = exp(score) written to attn_q1 bf16 (no max subtraction -
     |score| < 80 so fp32 exp cannot overflow); 6S = sixes.T @ E via
     matmul; attn_q1 *= 1/(6S) broadcast (folds softmax denom and the
     final /6). Two column-shifted copies attn_q0/attn_q2 are then made
     so every deconv window is CONTIGUOUS and can be a matmul weights
     operand (weights APs allow only one free dim).
  6. Deconv as polyphase conv-transpose, one chunk behind the scores
     loop, oriented output-major so no PE transposes are needed:
     psum[pix 128, rc 512] += attn_qw[:, kb, rows2, :].T @ kg[q,qw,kb][:, 2rh:2rh+2, :]
     over 18 shifts; drain = one scalar copy + 2 output DMAs (the sync
     engine issues descriptors serially at ~600ns, so DMA count -- not
     bytes -- is what the drain must minimize).

All matmuls use float32r (FP22 multiply, FP32 accumulate) or bf16 at
full PE rate. A bf16 warmup bridges the initial x-load DMA so the HAM
clock gate is released when real work starts.
"""

import numpy as np

_CACHE = {}

# Problem constants (hardcoded per harness contract)
B = 8
H = W = 64
C = 64
CI = 32
HS = WS = 16
N = 256          # HS*WS low-res positions
PH = 66          # padded attn spatial extent (64 + 1 halo each side)

# tap order for the packed scores operands: packs A,B hold 4 taps each
# on partition groups 0-3; pack C holds the 9th tap on partitions 0-31
TAPS = [(0, 0), (0, 1), (0, 2), (1, 0), (1, 1), (1, 2), (2, 0), (2, 1),
        (2, 2)]


def _build_nc():
    import concourse.bass as bass
    import concourse.tile as tile
    from concourse import bacc, mybir
    from concourse.masks import make_identity
    from contextlib import ExitStack

    F32 = mybir.dt.float32
    F32R = mybir.dt.float32r
    BF16 = mybir.dt.bfloat16
    Alu = mybir.AluOpType
    Act = mybir.ActivationFunctionType

    def r_(ap):
        return ap.bitcast(F32R)

    nc = bacc.Bacc("TRN2", debug=False)

    x_h = nc.dram_tensor("x", [H, W, C], F32, kind="ExternalInput")
    thw_h = nc.dram_tensor("theta_w", [C, CI], F32, kind="ExternalInput")
    thb_h = nc.dram_tensor("theta_b", [CI], F32, kind="ExternalInput")
    tha_h = nc.dram_tensor("theta_alpha", [CI], F32, kind="ExternalInput")
    phw_h = nc.dram_tensor("phi_w", [C, CI], F32, kind="ExternalInput")
    phb_h = nc.dram_tensor("phi_b", [CI], F32, kind="ExternalInput")
    pha_h = nc.dram_tensor("phi_alpha", [CI], F32, kind="ExternalInput")
    gw_h = nc.dram_tensor("g_w", [C, C], F32, kind="ExternalInput")
    gb_h = nc.dram_tensor("g_b", [C], F32, kind="ExternalInput")
    ga_h = nc.dram_tensor("g_alpha", [C], F32, kind="ExternalInput")
    y_h = nc.dram_tensor("y", [4 * H, 4 * W, C], F32, kind="ExternalOutput")

    with tile.TileContext(nc) as tc, ExitStack() as top:
        ec = top.enter_context

        consts = ec(tc.tile_pool(name="consts", bufs=1))
        persist = ec(tc.tile_pool(name="persist", bufs=1))
        phip = ec(tc.tile_pool(name="phip", bufs=1))
        dramp = ec(tc.tile_pool(name="dramp", bufs=1, space="DRAM"))
        staging = ec(tc.tile_pool(name="staging", bufs=3))
        ps_misc = ec(tc.tile_pool(name="ps_misc", bufs=2, space="PSUM"))
        ps_sc = ec(tc.tile_pool(name="ps_sc", bufs=2, space="PSUM"))
        ps_d = ec(tc.tile_pool(name="ps_d", bufs=3, space="PSUM"))

        # ---- constants / weights in SBUF ----
        ident = consts.tile([128, 128], F32)
        make_identity(nc, ident)
        # HAM warmup: bf16 matmuls keep the PE busy through the initial
        # x-load DMA so the clock gate is released when real work starts.
        wu = consts.tile([128, 512], BF16)
        nc.vector.memset(wu, 0.0)
        ps_wu = ps_sc.tile([128, 512], F32, tag="sc", name="ps_wu")
        for i in range(16):
            nc.tensor.matmul(ps_wu, wu[:, :128], wu, start=True, stop=True)
        thw_sb = consts.tile([C, CI], F32)
        nc.sync.dma_start(out=r_(thw_sb), in_=r_(thw_h.ap()))
        phw_sb = consts.tile([C, CI], F32)
        nc.sync.dma_start(out=r_(phw_sb), in_=r_(phw_h.ap()))
        gw_sb = consts.tile([C, C], F32)
        nc.sync.dma_start(out=r_(gw_sb), in_=r_(gw_h.ap()))
        # biases/alphas tiled 4x along partitions for the packed convs
        thb4 = consts.tile([128, 1], F32)
        tha4 = consts.tile([128, 1], F32)
        phb4 = consts.tile([128, 1], F32)
        pha4 = consts.tile([128, 1], F32)
        for g in range(4):
            sl = slice(32 * g, 32 * g + 32)
            nc.sync.dma_start(out=thb4[sl], in_=thb_h.ap().unsqueeze(1))
            nc.sync.dma_start(out=tha4[sl], in_=tha_h.ap().unsqueeze(1))
            nc.sync.dma_start(out=phb4[sl], in_=phb_h.ap().unsqueeze(1))
            nc.sync.dma_start(out=pha4[sl], in_=pha_h.ap().unsqueeze(1))
        gb_row = consts.tile([1, C], F32)
        nc.sync.dma_start(out=gb_row, in_=gb_h.ap().unsqueeze(0))
        ga_row = consts.tile([1, C], F32)
        nc.sync.dma_start(out=ga_row, in_=ga_h.ap().unsqueeze(0))
        gb_bc = consts.tile([128, C], F32)
        nc.gpsimd.partition_broadcast(gb_bc, gb_row)
        ga_bc = consts.tile([128, C], F32)
        nc.gpsimd.partition_broadcast(ga_bc, ga_row)
        # 1x1 conv weights replicated 4x along the output dim so the PE
        # emits (tap-group, ci) packed partitions directly
        thw4 = consts.tile([C, 128], F32)
        phw4 = consts.tile([C, 128], F32)
        for g in range(4):
            nc.vector.tensor_copy(out=r_(thw4[:, 32 * g:32 * g + 32]),
                                  in_=thw_sb)
            nc.vector.tensor_copy(out=r_(phw4[:, 32 * g:32 * g + 32]),
                                  in_=phw_sb)
        sixes128 = consts.tile([128, 1], BF16)
        nc.vector.memset(sixes128, 6.0)
        ones32 = consts.tile([CI, 1], F32)
        nc.vector.memset(ones32, 1.0)
        ztb16 = consts.tile([128, 1024], BF16)
        nc.vector.memset(ztb16, 0.0)

        # ---- persistent activation buffers ----
        # attn planes: 3 column-shifted bf16 copies, attn_q[qw] holding
        # logical padded columns (2-qw)..(66-qw) so every deconv window
        # [2 rows, 64 cols] is contiguous (valid as matmul weights)
        attn_q = []
        for qw in range(3):
            t_ = persist.tile([128, 2, PH, 64], BF16, tag=f"attnq{qw}",
                              name=f"attnq{qw}")
            nc.vector.memset(t_, 0.0)
            attn_q.append(t_)
        # packed theta operands: thPack[P][(g ci), R, C] =
        # theta_pad[ci, R + kh, C + kw] for tap (kh,kw) in group g
        thpA = persist.tile([128, 64, 64], F32)
        nc.vector.memset(thpA, 0.0)
        thpB = persist.tile([128, 64, 64], F32)
        nc.vector.memset(thpB, 0.0)
        thpC = persist.tile([CI, 64, 64], F32)
        nc.vector.memset(thpC, 0.0)
        thp = [thpA, thpB, thpC]
        # packed phi operands [(<=4 tap, ci), n]
        fpA = persist.tile([128, N], F32)
        fpB = persist.tile([128, N], F32)
        fpC = persist.tile([CI, N], F32)

        phi4_pad = phip.tile([128, 18, 18], F32)
        nc.vector.memset(phi4_pad, 0.0)
        n2p = phip.tile([1, 324], F32)
        nrm = phip.tile([1, N], F32)
        s10b = phip.tile([128, N], F32)

        # polyphase layout: g_poly[hq, wq, hr, (wr c)] = g[4hq+hr-4, 4wq+wr-4, c]
        g_poly = dramp.tile([18, 18, 4, 256], BF16)
        g_lin = dramp.tile([H, W, C], BF16)

        with ExitStack() as st1:
            e1 = st1.enter_context
            xt_pool = e1(tc.tile_pool(name="xt_pool", bufs=1))
            gsb_pool = e1(tc.tile_pool(name="gsb_pool", bufs=1))
            ttmp = e1(tc.tile_pool(name="ttmp", bufs=2))
            gtmp = e1(tc.tile_pool(name="gtmp", bufs=3))

            xP = xt_pool.tile([128, 32, C], F32)
            x_r = x_h.ap().rearrange("h w c -> (h w) c").rearrange(
                "(t p) c -> p t c", p=128)
            for xc in range(4):
                nc.sync.dma_start(
                    out=xP[:, xc * 8:(xc + 1) * 8, :],
                    in_=x_r[:, xc * 8:(xc + 1) * 8, :])
            xT = xt_pool.tile([C, H, W], F32)
            xTf = xT.rearrange("c h w -> c (h w)")
            phi_inT = xt_pool.tile([C, HS, WS], F32)
            g_sb = gsb_pool.tile([128, 32, C], BF16)

            def theta_chunk(ch):
                h0 = ch * 8
                ps_t = ps_misc.tile([128, 512], F32, tag="m",
                                    name=f"ps_t{ch}")
                nc.tensor.matmul(
                    ps_t, r_(thw4), r_(xTf[:, ch * 512:(ch + 1) * 512]),
                    start=True, stop=True)
                t_lin = ttmp.tile([128, 8, W], F32, tag="tl")
                nc.vector.tensor_scalar_add(
                    t_lin.rearrange("p a b -> p (a b)"), ps_t, thb4)
                t_neg = ttmp.tile([128, 8, W], F32, tag="tn")
                nc.vector.tensor_scalar(
                    t_neg.rearrange("p a b -> p (a b)"),
                    t_lin.rearrange("p a b -> p (a b)"),
                    0.0, tha4, Alu.min, Alu.mult)
                # bake each tap's spatial shift into its pack slice
                for t, (kh, kw) in enumerate(TAPS):
                    P, g = t // 4, t % 4
                    R0 = max(0, h0 + 1 - kh)
                    R1 = min(64, h0 + 9 - kh)
                    C0 = max(0, 1 - kw)
                    C1 = min(64, 65 - kw)
                    rs = R0 + kh - h0 - 1
                    cs = C0 + kw - 1
                    sl = slice(32 * g, 32 * g + 32)
                    nc.vector.scalar_tensor_tensor(
                        out=r_(thp[P][sl, R0:R1, C0:C1]),
                        in0=t_lin[sl, rs:rs + R1 - R0, cs:cs + C1 - C0],
                        scalar=0.0,
                        in1=t_neg[sl, rs:rs + R1 - R0, cs:cs + C1 - C0],
                        op0=Alu.max, op1=Alu.add)

            # interleaved: transposes -> g matmuls -> theta per x chunk
            for xc in range(4):
                for t in range(xc * 8, (xc + 1) * 8):
                    ps_x = ps_misc.tile([C, 128], F32, tag="m",
                                        name=f"ps_x{t}")
                    nc.tensor.transpose(ps_x, xP[:, t, :], ident)
                    nc.scalar.copy(
                        out=r_(xTf[:, t * 128:(t + 1) * 128]), in_=ps_x)
                for t in range(xc * 8, (xc + 1) * 8):
                    ps_g = ps_misc.tile([128, C], F32, tag="m",
                                        name=f"ps_g{t}")
                    nc.tensor.matmul(
                        ps_g, r_(xTf[:, t * 128:(t + 1) * 128]), r_(gw_sb),
                        start=True, stop=True)
                    gv = gtmp.tile([128, C], F32, tag="gv")
                    nc.vector.tensor_add(gv, ps_g, gb_bc)
                    gm1 = gtmp.tile([128, C], F32, tag="gm1")
                    nc.vector.tensor_scalar_max(gm1, gv, 0.0)
                    nc.vector.tensor_scalar_min(gv, gv, 0.0)
                    nc.vector.tensor_mul(gv, gv, ga_bc)
                    nc.vector.scalar_tensor_tensor(
                        out=g_sb[:, t, :], in0=gm1, scalar=1.0, in1=gv,
                        op0=Alu.mult, op1=Alu.add)
                theta_chunk(2 * xc)
                theta_chunk(2 * xc + 1)

            # g_sb -> g_lin [h, w, c], then 4 DRAM->DRAM repacks into the
            # polyphase interior (one per hr phase)
            glint = g_lin.rearrange("(t a) w c -> a w t c", a=2)
            for p1 in range(2):
                nc.sync.dma_start(
                    out=glint[p1], in_=g_sb[p1 * 64:(p1 + 1) * 64, :, :])
            gl5 = g_lin.rearrange("(hq hr) (wq wr) c -> hq hr wq (wr c)",
                                  hr=4, wr=4)
            for hr in range(4):
                nc.sync.dma_start(out=g_poly[1:17, 1:17, hr, :],
                                  in_=gl5[:, hr, :, :])

            # phi: bilinear downsample (4-tap avg), then the packed 1x1
            # conv + prelu into a 4x-replicated padded plane
            xv = xT.rearrange("c (hq hs) (wq ws) -> c hq hs wq ws", hs=4, ws=4)
            nc.vector.tensor_add(r_(phi_inT), xv[:, :, 1, :, 1],
                                 xv[:, :, 1, :, 2])
            nc.vector.tensor_add(r_(phi_inT), phi_inT, xv[:, :, 2, :, 1])
            nc.vector.tensor_add(r_(phi_inT), phi_inT, xv[:, :, 2, :, 2])
            nc.vector.tensor_scalar_mul(r_(phi_inT), phi_inT, 0.25)
            ps_phi = ps_misc.tile([128, N], F32, tag="m")
            nc.tensor.matmul(
                ps_phi, r_(phw4), r_(phi_inT.rearrange("c a b -> c (a b)")),
                start=True, stop=True)
            p_lin = ttmp.tile([128, HS, WS], F32, tag="pl")
            nc.vector.tensor_scalar_add(
                p_lin.rearrange("p a b -> p (a b)"), ps_phi, phb4)
            p_neg = ttmp.tile([128, HS, WS], F32, tag="pn")
            nc.vector.tensor_scalar(
                p_neg.rearrange("p a b -> p (a b)"),
                p_lin.rearrange("p a b -> p (a b)"),
                0.0, pha4, Alu.min, Alu.mult)
            nc.vector.scalar_tensor_tensor(
                out=phi4_pad[:, 1:17, 1:17],
                in0=p_lin, scalar=0.0, in1=p_neg,
                op0=Alu.max, op1=Alu.add)

            # per-patch L2 norm (group 0 holds a full phi copy)
            sq = ttmp.tile([CI, 324], F32, tag="sq")
            nc.scalar.activation(r_(sq),
                                 phi4_pad[:CI].rearrange("p a b -> p (a b)"),
                                 Act.Square)
            ps_n2 = ps_misc.tile([1, 324], F32, tag="m")
            nc.tensor.matmul(ps_n2, r_(ones32), r_(sq), start=True, stop=True)
            nc.scalar.copy(out=n2p, in_=ps_n2)
            n2v = n2p.rearrange("p (a b) -> p a b", b=18)
            nrm3 = nrm.rearrange("p (a b) -> p a b", b=WS)
            nc.vector.tensor_add(nrm3, n2v[:, 0:16, 0:16], n2v[:, 0:16, 1:17])
            for kh in range(3):
                for kw in range(3):
                    if kh == 0 and kw < 2:
                        continue
                    nc.vector.tensor_add(
                        nrm3, nrm3, n2v[:, kh:kh + 16, kw:kw + 16])
            nc.scalar.sqrt(nrm, nrm)
            nc.vector.tensor_scalar_max(nrm, nrm, 1e-6)
            nc.vector.reciprocal(nrm, nrm)
            nc.vector.tensor_scalar_mul(nrm, nrm, 10.0)
            nc.gpsimd.partition_broadcast(s10b, nrm)

            # extract phi tap packs (partition-aligned shifted windows),
            # with the softmax scale folded in
            fp_flat = [fpA, fpB]
            for t, (kh, kw) in enumerate(TAPS):
                P, g = t // 4, t % 4
                sl = slice(32 * g, 32 * g + 32)
                dst = fpC if P == 2 else fp_flat[P][sl]
                nc.vector.tensor_copy(
                    out=r_(dst.rearrange("p (a b) -> p a b", b=WS)),
                    in_=phi4_pad[sl, kh:kh + 16, kw:kw + 16])
            nc.vector.tensor_mul(r_(fpA), fpA, s10b)
            nc.vector.tensor_mul(r_(fpB), fpB, s10b)
            nc.vector.tensor_mul(r_(fpC), fpC, s10b[:CI])

        # ---- stage 2: fused scores/softmax/deconv pipeline ----
        with ExitStack() as st2:
            e2 = st2.enter_context
            kgp = e2(tc.tile_pool(name="kgp", bufs=1))
            rbp = e2(tc.tile_pool(name="rbp", bufs=3))
            schp = e2(tc.tile_pool(name="schp", bufs=2))

            # zero the g_poly border ring (conv-transpose SAME padding);
            # issued here so these DMAs never gate the stage-1 pipeline
            gp_r0 = g_poly[0].rearrange("b r x -> (b r x)").rearrange(
                "(p f) -> p f", f=512)
            nc.sync.dma_start(out=gp_r0, in_=ztb16[:36, :512])
            gp_r1 = g_poly[17].rearrange("b r x -> (b r x)").rearrange(
                "(p f) -> p f", f=512)
            nc.sync.dma_start(out=gp_r1, in_=ztb16[:36, :512])
            gp_c0 = g_poly[1:17, 0].rearrange("a r x -> a (r x)")
            nc.sync.dma_start(out=gp_c0, in_=ztb16[:16])
            gp_c1 = g_poly[1:17, 17].rearrange("a r x -> a (r x)")
            nc.sync.dma_start(out=gp_c1, in_=ztb16[:16])

            # gather the 18 dynamic-filter tiles from g_poly, one DMA each
            # kg[q,qw,kb][(i,j), r, (rw c)] = g_poly[i+kb*8+q, j+qw, r, :]
            kg = {}
            for q in range(3):
                for qw in range(3):
                    for kb in range(2):
                        t_ = kgp.tile([128, 4, 256], BF16,
                                      tag=f"kg{q}{qw}{kb}",
                                      name=f"kg{q}{qw}{kb}")
                        gsrc = g_poly[kb * 8 + q: kb * 8 + q + 8,
                                      qw: qw + 16, :, :]
                        nc.sync.dma_start(out=t_, in_=gsrc)
                        kg[(q, qw, kb)] = t_

            # y viewed as [hq, wq, r, (rw c)] for the output-major drain
            y_r2 = y_h.ap().rearrange(
                "(hq r) (wq rw) c -> hq wq r (rw c)", r=4, rw=4)

            def deconv_pc(pc):
                # output rows 8pc..8pc+8, in 4 row-pairs x 2 r-halves
                for pxc in range(4):
                    hp = 8 * pc + 2 * pxc
                    for rh in range(2):
                        ps_o = ps_d.tile([128, 512], F32, tag="d",
                                         name=f"ps_o{pc}_{pxc}_{rh}")
                        first = True
                        for q in range(3):
                            for qw in range(3):
                                for kb in range(2):
                                    nc.tensor.matmul(
                                        ps_o,
                                        attn_q[qw][:, kb,
                                                   hp + 2 - q:hp + 4 - q, :],
                                        kg[(q, qw, kb)][:,
                                                        2 * rh:2 * rh + 2, :],
                                        start=first,
                                        stop=(q == 2 and qw == 2 and kb == 1))
                                    first = False
                        st_ = staging.tile([128, 512], F32, tag="stg",
                                           name=f"st{pc}_{pxc}_{rh}")
                        nc.scalar.copy(out=st_, in_=ps_o)
                        st2v = st_.rearrange("p (r x) -> p r x", r=2)
                        for a in range(2):
                            hq = pc * 8 + pxc * 2 + a
                            nc.sync.dma_start(
                                out=y_r2[hq, :, 2 * rh:2 * rh + 2, :],
                                in_=st2v[a * 64:(a + 1) * 64])

            for ch in range(8):
                h0 = ch * 8
                # scoresT for both n-blocks via 3 packed matmuls, then
                # E = exp(score)
                for kb in range(2):
                    ps_s = ps_sc.tile([128, 512], F32, tag="sc",
                                      name=f"ps_s{ch}_{kb}")
                    nc.tensor.matmul(
                        ps_s, r_(fpA[:, kb * 128:(kb + 1) * 128]),
                        r_(thpA[:, h0:h0 + 8, :]),
                        start=True, stop=False)
                    nc.tensor.matmul(
                        ps_s, r_(fpB[:, kb * 128:(kb + 1) * 128]),
                        r_(thpB[:, h0:h0 + 8, :]),
                        start=False, stop=False)
                    nc.tensor.matmul(
                        ps_s, r_(fpC[:, kb * 128:(kb + 1) * 128]),
                        r_(thpC[:, h0:h0 + 8, :]),
                        start=False, stop=True)
                    nc.scalar.activation(
                        out=attn_q[1][:, kb, 1 + h0:9 + h0, :],
                        in_=ps_s.rearrange("p (a b) -> p a b", b=64),
                        func=Act.Exp)
                if ch >= 2:
                    deconv_pc(ch - 2)
                # 6S = sixes.T @ E (ones-matmul), rb = 1/(6S) broadcast
                ps_S = ps_misc.tile([1, 512], F32, tag="m", name=f"ps_S{ch}")
                for kb in range(2):
                    nc.tensor.matmul(
                        ps_S, sixes128,
                        attn_q[1][:, kb, 1 + h0:9 + h0, :],
                        start=(kb == 0), stop=(kb == 1))
                sch = schp.tile([1, 512], F32, tag="sch", name=f"sch{ch}")
                nc.vector.reciprocal(sch, ps_S)
                rb_t = rbp.tile([128, 512], F32, tag="rb", name=f"rb{ch}")
                nc.gpsimd.partition_broadcast(rb_t, sch)
                rb3 = rb_t.rearrange("p (a b) -> p a b", b=64)
                for kb in range(2):
                    nc.vector.tensor_mul(
                        attn_q[1][:, kb, 1 + h0:9 + h0, :],
                        attn_q[1][:, kb, 1 + h0:9 + h0, :], rb3)
                    # column-shifted copies for the deconv windows
                    nc.vector.tensor_copy(
                        out=attn_q[0][:, kb, 1 + h0:9 + h0, 0:63],
                        in_=attn_q[1][:, kb, 1 + h0:9 + h0, 1:64])
                    nc.vector.tensor_copy(
                        out=attn_q[2][:, kb, 1 + h0:9 + h0, 1:64],
                        in_=attn_q[1][:, kb, 1 + h0:9 + h0, 0:63])
            deconv_pc(6)
            deconv_pc(7)

    nc.finalize()
    return nc


def kernel(**inputs):
    from concourse.bass_utils import run_bass_kernel_spmd

    if "nc" not in _CACHE:
        _CACHE["nc"] = _build_nc()
    nc = _CACHE["nc"]

    arrs = {k: np.ascontiguousarray(np.asarray(v, dtype=np.float32))
            for k, v in inputs.items()}
    x = arrs.pop("x")
    in_maps = [dict(arrs, x=x[b]) for b in range(B)]
    res = run_bass_kernel_spmd(nc, in_maps, core_ids=list(range(B)))
    return np.stack([res.results[b]["y"] for b in range(B)])
